# revision 1
# baseline (speedup 1.0000x reference)
"""GCN classifier with metrics — TRN2 Bass kernel (8 NeuronCores, SPMD).

Strategy:
  - Nodes partitioned contiguously across 8 cores (12500/core, padded to 12544).
  - Per layer: h_scaled = (x @ W) * dinv[node] computed per-shard, AllGathered
    into a full bf16 table [100352, 128] (64 feats + 64 zero pad per row).
  - Edge aggregation: for each 128-dst-node tile, gather the incident edges'
    source rows (dma_gather, int16 window-relative indices over 4 windows of
    25088 rows) and contract with host-shipped one-hot selection matrices
    S [slot,dst] via PE matmul accumulate in PSUM:
       agg[d,:] = sum_k S[k,d] * msg[k,:]   (bf16 x bf16 -> f32 PSUM)
  - global_mean_pool: indicator matmul per tile into a PSUM accumulator over
    two 128-graph windows, AllReduce [256,17] across cores, head computed
    redundantly on every core.
"""
import sys
import numpy as np

sys.path.insert(0, "/opt/trn_rl_repo")

import ml_dtypes
import concourse.bass as bass
import concourse.bacc as bacc
import concourse.mybir as mybir
import concourse.tile as tile
from concourse.bass_utils import run_bass_kernel_spmd
from concourse.library_config import mlp as mlp_lib

BF16 = ml_dtypes.bfloat16

N = 100_000
E = 1_600_000
G = 256
CIN = 128
NCLS = 10
NCORES = 8
SHARD = 12_500
SHARD_PAD = 12_544          # 98 * 128
NT = 98                     # tiles per core
WIN = 25_088                # table rows per source window (2 shards)
NWIN = 4
TROWS = NCORES * SHARD_PAD  # 100352 table rows
H1 = 64
H2 = 16
GROUP_T = 4                 # dst tiles per group
MAX_CALL_BLK = 8            # blocks (128 slots) per dma_gather call (<=1024 idx)
F32 = mybir.dt.float32
BF = mybir.dt.bfloat16
I16 = mybir.dt.int16


def _wrap_idx(idx):
    """[n] int16 (n % 128 == 0) -> [128, n//16] wrapped + replicated layout."""
    n = len(idx)
    w = idx.reshape(n // 16, 16).T.astype(np.int16)   # [16, n/16]
    return np.tile(w, (8, 1))


def _build_structure(src, dst):
    """Shared (SPMD-uniform) slot structure + per-core index/S data.

    Returns (schedule, per_core_data):
      schedule: dict with group/call/block program structure (same all cores)
      per_core: list of dicts with idx16 [128, TOTCOL], S [128, TOTBLK*128] bf16
    """
    # table row of each global node
    node_row = (np.arange(N) // SHARD) * SHARD_PAD + (np.arange(N) % SHARD)

    # per-core edge lists (dst side), with self loops
    # edge (s -> d): core = d // SHARD
    ecore = dst // SHARD
    order = np.argsort(ecore, kind="stable")
    src_o, dst_o = src[order], dst[order]
    core_bounds = np.searchsorted(ecore[order], np.arange(NCORES + 1))

    # slot lists per (core, tile, window): row-relative idx + local partition
    counts = np.zeros((NCORES, NT, NWIN), np.int64)
    per_core_slots = []
    for c in range(NCORES):
        s_c = src_o[core_bounds[c]:core_bounds[c + 1]]
        d_c = dst_o[core_bounds[c]:core_bounds[c + 1]]
        # add self loops for real nodes of this core
        own = np.arange(c * SHARD, (c + 1) * SHARD)
        s_all = np.concatenate([s_c, own])
        d_all = np.concatenate([d_c, own])
        dloc = d_all - c * SHARD                     # [0, 12500)
        t_all = dloc // 128
        p_all = dloc % 128
        rows = node_row[s_all]
        w_all = rows // WIN
        rel = rows - w_all * WIN
        # sort by (tile, window, rel) for locality
        key = (t_all * NWIN + w_all) * (WIN + 1) + rel
        o2 = np.argsort(key, kind="stable")
        t_all, w_all, rel, p_all = t_all[o2], w_all[o2], rel[o2], p_all[o2]
        tw = t_all * NWIN + w_all
        cnt = np.bincount(tw, minlength=NT * NWIN).reshape(NT, NWIN)
        counts[c] = cnt
        bounds = np.concatenate([[0], np.cumsum(cnt.ravel())])
        per_core_slots.append((rel.astype(np.int32), p_all.astype(np.int32), bounds))

    maxc = counts.max(axis=0)                        # [NT, NWIN]
    nblk = (maxc + 127) // 128                       # blocks per (t, w)
    nblk = np.maximum(nblk, (maxc > 0).astype(np.int64))

    # global block/call schedule, grouped
    groups = []
    blk_off = 0
    col_off = 0
    totblk = int(nblk.sum())
    totslot = totblk * 128
    for g0 in range(0, NT, GROUP_T):
        tiles = list(range(g0, min(NT, g0 + GROUP_T)))
        calls = []
        blocks_of_tile = {t: [] for t in tiles}
        for w in range(NWIN):
            # block list for this (group, window): [(tile, blk_within)]
            wblocks = []
            for t in tiles:
                for b in range(int(nblk[t, w])):
                    wblocks.append(t)
            # split into calls of <= MAX_CALL_BLK blocks
            i = 0
            while i < len(wblocks):
                chunk = wblocks[i:i + MAX_CALL_BLK]
                call = {
                    "w": w,
                    "nb": len(chunk),
                    "col": col_off,            # idx column offset (int16 cols)
                    "blk": blk_off,            # global block index of first block
                    "tiles": chunk,            # tile of each block
                }
                calls.append(call)
                for j, t in enumerate(chunk):
                    blocks_of_tile[t].append((blk_off + j, call))
                blk_off += len(chunk)
                col_off += len(chunk) * 8      # 128 idx / 16 per col
                i += MAX_CALL_BLK
        groups.append({"tiles": tiles, "calls": calls,
                       "blocks_of_tile": blocks_of_tile})
    assert blk_off == totblk

    # per-core idx + S data following the global block order
    per_core = []
    for c in range(NCORES):
        rel, part, bounds = per_core_slots[c]
        idx_cols = np.zeros((128, col_off), np.int16)
        S = np.zeros((128, totblk * 128), BF16)
        for g in groups:
            for call in g["calls"]:
                w = call["w"]
                # build the call's slot stream: per block -> (tile, w) slots
                stream = np.zeros(call["nb"] * 128, np.int16)
                # track position within each tile's (t,w) run
                for j, t in enumerate(call["tiles"]):
                    # which block of (t, w) is this within the call sequence?
                    # blocks of (t,w) appear consecutively across calls in order
                    pass
                # simpler: fill per (t,w) runs below
                call["_stream"] = stream
            # fill streams per (t, w)
            for t in g["tiles"]:
                for w in range(NWIN):
                    nb_tw = int(nblk[t, w])
                    if nb_tw == 0:
                        continue
                    lo = bounds[t * NWIN + w]
                    hi = bounds[t * NWIN + w + 1]
                    r = rel[lo:hi]
                    p = part[lo:hi]
                    nsl = nb_tw * 128
                    rr = np.zeros(nsl, np.int32)
                    pp = np.full(nsl, -1, np.int32)
                    rr[:hi - lo] = r
                    pp[:hi - lo] = p
                    # locate this (t,w)'s blocks in the calls
                    k = 0
                    for bidx, call in g["blocks_of_tile"][t]:
                        if call["w"] != w:
                            continue
                        off_in_call = (bidx - call["blk"]) * 128
                        seg_r = rr[k * 128:(k + 1) * 128]
                        seg_p = pp[k * 128:(k + 1) * 128]
                        call["_stream"][off_in_call:off_in_call + 128] = seg_r
                        valid = seg_p >= 0
                        S[np.nonzero(valid)[0], bidx * 128 + seg_p[valid]] = 1
                        k += 1
                    assert k == nb_tw
            for call in g["calls"]:
                wrapped = _wrap_idx(call["_stream"])
                idx_cols[:, call["col"]:call["col"] + call["nb"] * 8] = wrapped
                del call["_stream"]
        per_core.append({"idx": idx_cols, "S": S})

    sched = {"groups": groups, "totblk": totblk, "totcol": col_off,
             "nblk": nblk}
    return sched, per_core


def _build_program(sched):
    nc = bacc.Bacc("TRN2", target_bir_lowering=False, debug=False,
                   num_devices=NCORES, num_swdge_queues=4)
    totblk, totcol = sched["totblk"], sched["totcol"]

    def inp(name, shape, dt=F32):
        return nc.declare_dram_parameter(name, shape, dt, isOutput=False)

    xs = inp("xs", [SHARD_PAD, CIN])
    dinv = inp("dinv", [128, NT])
    batchf = inp("batchf", [128, NT])
    iota01 = inp("iota01", [128, 256])
    ident = inp("ident", [128, 128])
    idxT = inp("idx", [128, totcol], I16)
    S_dram = inp("S", [128, totblk * 128], BF)
    W1 = inp("W1", [CIN, H1]);  Wr1 = inp("Wr1", [CIN, H1])
    W2 = inp("W2", [H1, H2]);   Wr2 = inp("Wr2", [H1, H2])
    b1b = inp("b1b", [128, H1]); br1b = inp("br1b", [128, H1])
    b2b = inp("b2b", [128, H2]); br2b = inp("br2b", [128, H2])
    Wf1t = inp("Wf1t", [16, 80]); Wf1b = inp("Wf1b", [H1, 80])
    Wf2 = inp("Wf2", [80, NCLS])
    bf1r = inp("bf1r", [1, 80]); bf2r = inp("bf2r", [1, NCLS])
    mcin = inp("mcin", [1, 80])
    alpha = inp("alpha", [128, 2])   # col0 = alpha1, col1 = alpha2 broadcast
    out = nc.declare_dram_parameter("out", [G, NCLS], F32, isOutput=True)

    SILU = mybir.ActivationFunctionType.Silu

    with tile.TileContext(nc) as tc:
        with tc.tile_pool(name="const", bufs=1) as constp, \
             tc.tile_pool(name="store", bufs=1) as storep, \
             tc.tile_pool(name="xp", bufs=4) as xp, \
             tc.tile_pool(name="hp", bufs=4) as hp, \
             tc.tile_pool(name="msg", bufs=26) as msgp, \
             tc.tile_pool(name="stl", bufs=26) as stlp, \
             tc.tile_pool(name="idxp", bufs=24) as idxp, \
             tc.tile_pool(name="ep", bufs=4) as ep, \
             tc.tile_pool(name="dram", bufs=1, space="DRAM") as dram, \
             tc.tile_pool(name="ps_tp", bufs=1, space="PSUM") as ps_tp, \
             tc.tile_pool(name="ps_mm", bufs=1, space="PSUM") as ps_mm, \
             tc.tile_pool(name="ps_agg", bufs=5, space="PSUM") as ps_agg, \
             tc.tile_pool(name="ps_pool", bufs=1, space="PSUM") as ps_pool:

            nc.gpsimd.load_library(mlp_lib)

            # ---- resident constants ----
            def ld(ap_src, shape, dt=F32, tag=None):
                t = constp.tile(shape, dt, tag=tag or ap_src.tensor.name)
                nc.sync.dma_start(out=t[:], in_=ap_src)
                return t

            dinv_sb = ld(dinv[:], [128, NT])
            batch_sb = ld(batchf[:], [128, NT])
            iota_sb = ld(iota01[:], [128, 256])
            ident_sb = ld(ident[:], [128, 128])
            W1_sb = ld(W1[:], [CIN, H1]); Wr1_sb = ld(Wr1[:], [CIN, H1])
            W2_sb = ld(W2[:], [H1, H2]); Wr2_sb = ld(Wr2[:], [H1, H2])
            b1_sb = ld(b1b[:], [128, H1]); br1_sb = ld(br1b[:], [128, H1])
            b2_sb = ld(b2b[:], [128, H2]); br2_sb = ld(br2b[:], [128, H2])
            Wf1t_sb = ld(Wf1t[:], [16, 80]); Wf1b_sb = ld(Wf1b[:], [H1, 80])
            Wf2_sb = ld(Wf2[:], [80, NCLS])
            bf1_sb = ld(bf1r[:], [1, 80]); bf2_sb = ld(bf2r[:], [1, NCLS])
            al_sb = ld(alpha[:], [128, 2])
            ones1 = constp.tile([1, 128], F32, tag="ones1")
            nc.vector.memset(ones1[:], 1.0)

            r1_store = storep.tile([128, NT * H1], F32, tag="r1s")
            r2_store = storep.tile([128, NT * H2], F32, tag="r2s")

            h1s_shard = dram.tile([SHARD_PAD, 128], BF)
            table1 = dram.tile([TROWS, 128], BF)
            h2s_shard = dram.tile([SHARD_PAD, 128], BF)
            table2 = dram.tile([TROWS, 128], BF)
            pool_in = dram.tile([G, 17], F32)
            pool_out = dram.tile([G, 17], F32)

            # ---------------- stage 0: h1s shard + r1 ----------------
            XB = 4
            for t0 in range(0, NT, XB):
                nt = min(XB, NT - t0)
                xw = xp.tile([128, XB * CIN], F32, tag="xw")
                nc.scalar.dma_start(
                    out=xw[:, :nt * CIN],
                    in_=xs[t0 * 128:(t0 + nt) * 128, :].rearrange(
                        "(a p) c -> p a c", p=128))
                h1w = hp.tile([128, XB * 128], BF, tag="h1w")
                for a in range(nt):
                    t = t0 + a
                    xT_ps = ps_tp.tile([128, 128], F32, tag="tp")
                    nc.tensor.transpose(out=xT_ps[:], in_=xw[:, a * CIN:(a + 1) * CIN],
                                        identity=ident_sb[:])
                    xT = xp.tile([128, 128], F32, tag="xT")
                    nc.vector.tensor_copy(out=xT[:], in_=xT_ps[:])

                    hpre = ps_mm.tile([128, H1], F32, tag="mm")
                    nc.tensor.matmul(out=hpre[:], lhsT=xT[:], rhs=W1_sb[:],
                                     start=True, stop=True)
                    nc.vector.memset(h1w[:, a * 128 + H1:(a + 1) * 128], 0.0)
                    nc.vector.tensor_scalar_mul(
                        out=h1w[:, a * 128:a * 128 + H1], in0=hpre[:],
                        scalar1=dinv_sb[:, t:t + 1])

                    r1ps = ps_mm.tile([128, H1], F32, tag="mm")
                    nc.tensor.matmul(out=r1ps[:], lhsT=xT[:], rhs=Wr1_sb[:],
                                     start=True, stop=True)
                    r1a = ep.tile([128, H1], F32, tag="r1a")
                    nc.vector.tensor_add(out=r1a[:], in0=r1ps[:], in1=br1_sb[:])
                    nc.scalar.activation(out=r1a[:], in_=r1a[:], func=SILU)
                    nc.vector.tensor_scalar_mul(
                        out=r1_store[:, t * H1:(t + 1) * H1], in0=r1a[:],
                        scalar1=al_sb[:, 0:1])
                nc.scalar.dma_start(
                    out=h1s_shard[t0 * 128:(t0 + nt) * 128, :].rearrange(
                        "(a p) c -> p a c", p=128),
                    in_=h1w[:, :nt * 128])

            nc.gpsimd.collective_compute(
                "AllGather", mybir.AluOpType.bypass,
                replica_groups=[list(range(NCORES))],
                ins=[h1s_shard.opt()], outs=[table1.opt()])

            qctr = [0]

            def run_groups(table, epilogue):
                for g in sched["groups"]:
                    aggs = {}
                    for t in g["tiles"]:
                        aggs[t] = ps_agg.tile([128, H1], F32, tag="agg", name=f"agg{t}")
                    first = {t: True for t in g["tiles"]}
                    nb_left = {t: sum(1 for _ in g["blocks_of_tile"][t])
                               for t in g["tiles"]}
                    for call in g["calls"]:
                        nb = call["nb"]
                        it = idxp.tile([128, MAX_CALL_BLK * 8], I16, tag="it")
                        nc.scalar.dma_start(
                            out=it[:, :nb * 8],
                            in_=idxT[:, call["col"]:call["col"] + nb * 8])
                        mt = msgp.tile([128, MAX_CALL_BLK * 128], BF, tag="mt")
                        st = stlp.tile([128, MAX_CALL_BLK * 128], BF, tag="st")
                        nc.sync.dma_start(
                            out=st[:, :nb * 128],
                            in_=S_dram[:, call["blk"] * 128:
                                       (call["blk"] + nb) * 128])
                        w = call["w"]
                        nc.gpsimd.dma_gather(
                            mt[:, :nb * 128].rearrange("p (b d) -> p b d", d=128),
                            table[w * WIN:(w + 1) * WIN, :],
                            it[:, :nb * 8],
                            nb * 128, nb * 128, 128,
                            queue_num=qctr[0] % 4,
                        )
                        qctr[0] += 1
                        for j, t in enumerate(call["tiles"]):
                            nb_left[t] -= 1
                            nc.tensor.matmul(
                                out=aggs[t][:],
                                lhsT=st[:, j * 128:(j + 1) * 128],
                                rhs=mt[:, j * 128:j * 128 + H1],
                                start=first[t], stop=(nb_left[t] == 0))
                            first[t] = False
                    ntl = len(g["tiles"])
                    for pos, t in enumerate(g["tiles"]):
                        epilogue(t, aggs[t], pos, ntl)

            # ---------------- stage 1 ----------------
            h2w_box = [None]

            def epi1(t, agg, pos, ntl):
                a = ep.tile([128, H1], F32, tag="e1a")
                nc.vector.tensor_scalar_mul(out=a[:], in0=agg[:],
                                            scalar1=dinv_sb[:, t:t + 1])
                nc.vector.tensor_add(out=a[:], in0=a[:], in1=b1_sb[:])
                nc.scalar.activation(out=a[:], in_=a[:], func=SILU)
                h = ep.tile([128, H1], F32, tag="e1h")
                nc.vector.tensor_add(out=h[:], in0=a[:],
                                     in1=r1_store[:, t * H1:(t + 1) * H1])
                if pos == 0:
                    h2w_box[0] = hp.tile([128, GROUP_T * 128], BF, tag="h2w",
                                         name=f"h2w{t}")
                h2w = h2w_box[0]
                nc.vector.memset(h2w[:, pos * 128 + H1:(pos + 1) * 128], 0.0)
                nc.vector.tensor_scalar_mul(out=h2w[:, pos * 128:pos * 128 + H1],
                                            in0=h[:],
                                            scalar1=dinv_sb[:, t:t + 1])
                if pos == ntl - 1:
                    t0g = t - pos
                    nc.scalar.dma_start(
                        out=h2s_shard[t0g * 128:(t + 1) * 128, :].rearrange(
                            "(a p) c -> p a c", p=128),
                        in_=h2w[:, :ntl * 128])
                hT_ps = ps_tp.tile([128, 128], F32, tag="tp")
                nc.tensor.transpose(out=hT_ps[:H1, :], in_=h[:],
                                    identity=ident_sb[:])
                hT = ep.tile([H1, 128], F32, tag="e1ht")
                nc.vector.tensor_copy(out=hT[:], in_=hT_ps[:H1, :])
                r2ps = ps_mm.tile([128, H2], F32, tag="mm")
                nc.tensor.matmul(out=r2ps[:], lhsT=hT[:], rhs=Wr2_sb[:],
                                 start=True, stop=True)
                r2a = ep.tile([128, H2], F32, tag="e1r2")
                nc.vector.tensor_add(out=r2a[:], in0=r2ps[:], in1=br2_sb[:])
                nc.scalar.activation(out=r2a[:], in_=r2a[:], func=SILU)
                nc.vector.tensor_scalar_mul(
                    out=r2_store[:, t * H2:(t + 1) * H2], in0=r2a[:],
                    scalar1=al_sb[:, 1:2])

            run_groups(table1, epi1)

            nc.gpsimd.collective_compute(
                "AllGather", mybir.AluOpType.bypass,
                replica_groups=[list(range(NCORES))],
                ins=[h2s_shard.opt()], outs=[table2.opt()])

            # ---------------- stage 2 + pooling ----------------
            pool_ps = ps_pool.tile([128, 34], F32, tag="pool")
            nc.vector.memset(pool_ps[:], 0.0)
            tcount = [0]

            def epi2(t, agg, pos, ntl):
                a = ep.tile([128, H1], F32, tag="e2a")
                nc.vector.tensor_scalar_mul(out=a[:], in0=agg[:],
                                            scalar1=dinv_sb[:, t:t + 1])
                aT_ps = ps_tp.tile([128, 128], F32, tag="tp")
                nc.tensor.transpose(out=aT_ps[:H1, :], in_=a[:],
                                    identity=ident_sb[:])
                aT = ep.tile([H1, 128], F32, tag="e2at")
                nc.vector.tensor_copy(out=aT[:], in_=aT_ps[:H1, :])
                zps = ps_mm.tile([128, H2], F32, tag="mm")
                nc.tensor.matmul(out=zps[:], lhsT=aT[:], rhs=W2_sb[:],
                                 start=True, stop=True)
                zext = ep.tile([128, H2 + 1], F32, tag="e2z")
                nc.vector.tensor_add(out=zext[:, :H2], in0=zps[:], in1=b2_sb[:])
                nc.vector.tensor_add(out=zext[:, :H2], in0=zext[:, :H2],
                                     in1=r2_store[:, t * H2:(t + 1) * H2])
                nc.vector.memset(zext[:, H2:], 1.0)
                s0 = ep.tile([128, 128], F32, tag="e2s0")
                nc.vector.tensor_tensor(
                    out=s0[:], in0=batch_sb[:, t:t + 1].to_broadcast([128, 128]),
                    in1=iota_sb[:, 0:128], op=mybir.AluOpType.is_equal)
                k = tcount[0]
                nc.tensor.matmul(out=pool_ps[:, 0:17], lhsT=s0[:], rhs=zext[:],
                                 start=False, stop=(k == NT - 1),
                                 skip_group_check=True)
                s1 = ep.tile([128, 128], F32, tag="e2s1")
                nc.vector.tensor_tensor(
                    out=s1[:], in0=batch_sb[:, t:t + 1].to_broadcast([128, 128]),
                    in1=iota_sb[:, 128:256], op=mybir.AluOpType.is_equal)
                nc.tensor.matmul(out=pool_ps[:, 17:34], lhsT=s1[:], rhs=zext[:],
                                 start=False, stop=(k == NT - 1),
                                 skip_group_check=True)
                tcount[0] += 1

            run_groups(table2, epi2)

            psums = ep.tile([128, 34], F32, tag="psums")
            nc.vector.tensor_copy(out=psums[:], in_=pool_ps[:])
            nc.sync.dma_start(out=pool_in[0:128, :], in_=psums[:, 0:17])
            nc.sync.dma_start(out=pool_in[128:256, :], in_=psums[:, 17:34])

            nc.gpsimd.collective_compute(
                "AllReduce", mybir.AluOpType.add,
                replica_groups=[list(range(NCORES))],
                ins=[pool_in.opt()], outs=[pool_out.opt()])

            mc = ep.tile([1, 80], F32, tag="mmc")
            nc.sync.dma_start(out=mc[:], in_=mcin[:])

            # ---------------- classifier head (two graph windows) ----------
            for wdw in range(2):
                sums = ep.tile([128, 17], F32, tag="hsum")
                nc.sync.dma_start(out=sums[:],
                                  in_=pool_out[wdw * 128:(wdw + 1) * 128, :])
                cnt = ep.tile([128, 1], F32, tag="hcnt")
                nc.vector.tensor_scalar_max(out=cnt[:], in0=sums[:, 16:17],
                                            scalar1=1.0)
                rec = ep.tile([128, 1], F32, tag="hrec")
                nc.vector.reciprocal(out=rec[:], in_=cnt[:])
                ge = ep.tile([128, 16], F32, tag="hge")
                nc.vector.tensor_scalar_mul(out=ge[:], in0=sums[:, :16],
                                            scalar1=rec[:])
                geT_ps = ps_tp.tile([128, 128], F32, tag="tp")
                nc.tensor.transpose(out=geT_ps[:16, :], in_=ge[:],
                                    identity=ident_sb[:])
                geT = ep.tile([16, 128], F32, tag="hget")
                nc.vector.tensor_copy(out=geT[:], in_=geT_ps[:16, :])
                u_ps = ps_mm.tile([128, 80], F32, tag="mm")
                nc.tensor.matmul(out=u_ps[:], lhsT=geT[:], rhs=Wf1t_sb[:],
                                 start=True, stop=False)
                nc.tensor.matmul(out=u_ps[:], lhsT=ones1[:], rhs=mc[:],
                                 start=False, stop=True)
                u = ep.tile([128, 80], F32, tag="hu")
                nc.scalar.activation(out=u[:], in_=u_ps[:], func=SILU)
                uT_ps = ps_tp.tile([128, 128], F32, tag="tp")
                nc.tensor.transpose(out=uT_ps[:80, :], in_=u[:],
                                    identity=ident_sb[:])
                uT = ep.tile([80, 128], F32, tag="hut")
                nc.vector.tensor_copy(out=uT[:], in_=uT_ps[:80, :])
                o_ps = ps_mm.tile([128, NCLS], F32, tag="mm")
                nc.tensor.matmul(out=o_ps[:], lhsT=uT[:], rhs=Wf2_sb[:],
                                 start=True, stop=False)
                nc.tensor.matmul(out=o_ps[:], lhsT=ones1[:], rhs=bf2_sb[:],
                                 start=False, stop=True)
                o = ep.tile([128, NCLS], F32, tag="ho")
                nc.vector.tensor_copy(out=o[:], in_=o_ps[:])
                nc.sync.dma_start(out=out[wdw * 128:(wdw + 1) * 128, :],
                                  in_=o[:])

    nc.compile()
    return nc


def _host_metrics_contrib(tolerance, cost, time, quantity,
                          mW1, mb1, mW2, mb2, Wf1, bf1):
    silu = lambda v: v / (1.0 + np.exp(-v))
    m = np.stack([np.asarray(v, np.float32).reshape(1, 1) for v in
                  (tolerance, cost, time, quantity)])         # [4,1,1]
    e = silu(np.einsum('gij,gjk->gik', m, np.asarray(mW1, np.float32))
             + np.asarray(mb1, np.float32)[:, None, :])
    e = (np.einsum('gij,gjk->gik', e, np.asarray(mW2, np.float32))
         + np.asarray(mb2, np.float32)[:, None, :])           # [4,1,16]
    metvec = e.transpose(1, 0, 2).reshape(1, 64)
    mc = metvec @ np.asarray(Wf1, np.float32)[16:, :] + np.asarray(bf1, np.float32)[None, :]
    return mc.astype(np.float32)


def kernel(x, edge_index, batch, tolerance, cost, time, quantity,
           W1, b1, W2, b2, Wr1, br1, Wr2, br2, alpha1, alpha2,
           mW1, mb1, mW2, mb2, Wf1, bf1, Wf2, bf2):
    x = np.asarray(x, np.float32)
    src = np.asarray(edge_index[0], np.int64).astype(np.int64)
    dst = np.asarray(edge_index[1], np.int64).astype(np.int64)
    batch = np.asarray(batch, np.int64)

    deg = 1.0 + np.bincount(dst, minlength=N).astype(np.float32)
    dinv_full = 1.0 / np.sqrt(deg)

    sched, per_core = _build_structure(src.astype(np.int64), dst)
    nc = _build_program(sched)

    iota01 = np.tile(np.arange(256, dtype=np.float32), (128, 1))
    ident = np.eye(128, dtype=np.float32)
    common = {
        "iota01": iota01, "ident": ident,
        "W1": np.asarray(W1, np.float32), "Wr1": np.asarray(Wr1, np.float32),
        "W2": np.asarray(W2, np.float32), "Wr2": np.asarray(Wr2, np.float32),
        "b1b": np.tile(np.asarray(b1, np.float32), (128, 1)),
        "br1b": np.tile(np.asarray(br1, np.float32), (128, 1)),
        "b2b": np.tile(np.asarray(b2, np.float32), (128, 1)),
        "br2b": np.tile(np.asarray(br2, np.float32), (128, 1)),
        "Wf1t": np.asarray(Wf1[:16, :], np.float32),
        "Wf1b": np.asarray(Wf1[16:, :], np.float32),
        "Wf2": np.asarray(Wf2, np.float32),
        "bf1r": np.asarray(bf1, np.float32)[None, :],
        "bf2r": np.asarray(bf2, np.float32)[None, :],
        "mcin": _host_metrics_contrib(tolerance, cost, time, quantity,
                                      mW1, mb1, mW2, mb2, Wf1, bf1),
        "alpha": np.tile(np.array([[float(alpha1), float(alpha2)]],
                                  np.float32), (128, 1)),
    }

    in_maps = []
    for c in range(NCORES):
        lo, hi = c * SHARD, (c + 1) * SHARD
        xs = np.zeros((SHARD_PAD, CIN), np.float32)
        xs[:SHARD] = x[lo:hi]
        dv = np.zeros(SHARD_PAD, np.float32)
        dv[:SHARD] = dinv_full[lo:hi]
        bf_loc = np.full(SHARD_PAD, -1.0, np.float32)
        bf_loc[:SHARD] = batch[lo:hi].astype(np.float32)
        m = dict(common)
        m["xs"] = xs
        m["dinv"] = dv.reshape(NT, 128).T.copy()
        m["batchf"] = bf_loc.reshape(NT, 128).T.copy()
        m["idx"] = per_core[c]["idx"]
        m["S"] = per_core[c]["S"]
        in_maps.append(m)

    res = run_bass_kernel_spmd(nc, in_maps, list(range(NCORES)))
    kernel._last = (nc, in_maps)   # for external profiling harnesses
    kernel._res = res
    return np.asarray(res.results[0]["out"], np.float32)



# revision 7
# speedup vs baseline: 1.0366x; 1.0366x over previous
"""GCN classifier with metrics — TRN2 Bass kernel (8 NeuronCores, SPMD).

Strategy (v2):
  - Nodes partitioned contiguously across 8 cores (12500/core, padded to
    12544 = 98 tiles). Table rows are QUARTER-INTERLEAVED: window w holds
    quarter w of every core's shard, so the AllGather for each layer splits
    into 4 sub-AllGathers that overlap with gather/aggregate compute.
  - Per layer: h_scaled = (x @ W) * dinv[src] per-shard (bf16), 4
    sub-AllGathers build the replicated table [100352, 128] bf16
    (64 feats + 64 zero pad per row, 256B rows for dma_gather).
  - Edge aggregation, window-major: per (window, dst-tile) run, gather the
    incident edges' source rows (dma_gather, int16 window-relative indices,
    resident in SBUF) in calls of up to 8 blocks x 128 slots; per block a
    one-hot S [slot, dst] is generated ON-CHIP (vector is_equal against an
    iota row from a resident per-block dst-partition column) and contracted
    on the PE into a per-run PSUM accumulator, then added into a per-tile
    SBUF f32 accumulator. Self-loops never enter the edge list: they are
    folded into the accumulator init acc = (x@W) * dinv^2.
  - global_mean_pool: indicator matmul per tile into a PSUM accumulator over
    two 128-graph windows, AllReduce [256,17] across cores, head computed
    redundantly on every core.
"""
import sys
import numpy as np

sys.path.insert(0, "/opt/trn_rl_repo")

import ml_dtypes
import concourse.bass as bass
import concourse.bacc as bacc
import concourse.mybir as mybir
import concourse.tile as tile
from concourse.bass_utils import run_bass_kernel_spmd
from concourse.library_config import mlp as mlp_lib

BF16 = ml_dtypes.bfloat16

N = 100_000
E = 1_600_000
G = 256
CIN = 128
NCLS = 10
NCORES = 8
SHARD = 12_500
SHARD_PAD = 12_544          # 98 * 128
NT = 98                     # tiles per core
H1 = 64
H2 = 16
MAXBLK = 8                  # blocks (128 slots) per dma_gather call
NW = 4                      # source windows
Q_TILES = [25, 25, 25, 23]          # tiles per local quarter
STRIPE = [3200, 3200, 3200, 2944]   # rows per (core, quarter)
WROWS = [25600, 25600, 25600, 23552]
WBASE = [0, 25600, 51200, 76800]
QT0 = [0, 25, 50, 75]               # first tile of each quarter

F32 = mybir.dt.float32
BF = mybir.dt.bfloat16
I16 = mybir.dt.int16


def _wrap_idx(idx):
    """[n] int16 (n % 128 == 0) -> [128, n//16] wrapped + replicated layout."""
    n = len(idx)
    w = idx.reshape(n // 16, 16).T.astype(np.int16)   # [16, n/16]
    return np.tile(w, (8, 1))


def _build_structure(src, dst):
    """Slot structure shared by both layers.

    Returns (sched, per_core):
      sched: blocks [(w, t, first, last)], calls [(blk0, nb, w)], totblk
      per_core: list of dicts with idx [128, totblk*8] int16,
                pcol [128, totblk] bf16
    """
    g = np.arange(N, dtype=np.int64)
    c_of = g // SHARD
    i_of = g % SHARD
    q_of = np.minimum(i_of // 3200, 3)
    o_of = i_of - q_of * 3200
    stripe = np.array(STRIPE, dtype=np.int64)
    node_win = q_of.astype(np.int64)
    node_rel = o_of + c_of * stripe[q_of]     # row within window

    ecore = dst // SHARD
    order0 = np.argsort(ecore, kind="stable")
    src_o, dst_o = src[order0], dst[order0]
    cbounds = np.searchsorted(ecore[order0], np.arange(NCORES + 1))

    counts = np.zeros((NCORES, NW, NT), np.int64)
    percore_sorted = []
    for c in range(NCORES):
        s_c = src_o[cbounds[c]:cbounds[c + 1]]
        d_c = dst_o[cbounds[c]:cbounds[c + 1]]
        dloc = d_c - c * SHARD
        t = dloc // 128
        p = dloc % 128
        w = node_win[s_c]
        rel = node_rel[s_c]
        o2 = np.lexsort((rel, t, w))
        t, p, w, rel = t[o2], p[o2], w[o2], rel[o2]
        key = w * NT + t
        counts[c] = np.bincount(key, minlength=NW * NT).reshape(NW, NT)
        percore_sorted.append((key, rel, p))

    maxc = counts.max(axis=0)                         # [NW, NT]
    assert (maxc > 0).all()
    nblk = (maxc + 127) // 128                        # [NW, NT]

    blocks = []          # (w, t, first_of_run, last_of_run)
    run_blk_start = np.zeros(NW * NT, np.int64)
    acc_b = 0
    for w in range(NW):
        for t in range(NT):
            b = int(nblk[w, t])
            run_blk_start[w * NT + t] = acc_b
            for j in range(b):
                blocks.append((w, t, j == 0, j == b - 1))
            acc_b += b
    totblk = acc_b

    calls = []
    i = 0
    while i < totblk:
        w = blocks[i][0]
        nb = 1
        while nb < MAXBLK and i + nb < totblk and blocks[i + nb][0] == w:
            nb += 1
        calls.append((i, nb, w))
        i += nb

    per_core = []
    for c in range(NCORES):
        key, rel, p = percore_sorted[c]
        idxs = np.zeros(totblk * 128, np.int16)
        pc = np.full((128, totblk), 999.0, np.float32)
        grp_first = np.searchsorted(key, np.arange(NW * NT), side="left")
        ranks = np.arange(len(key)) - grp_first[key]
        slot = run_blk_start[key] * 128 + ranks
        idxs[slot] = rel.astype(np.int16)
        pc[slot % 128, slot // 128] = p
        per_core.append({"idx": _wrap_idx(idxs),
                         "pcol": pc.astype(BF16)})

    sched = {"blocks": blocks, "calls": calls, "totblk": totblk,
             "nblk": nblk}
    return sched, per_core


def _quarter_chunks():
    """Tile chunks (<=4 tiles) that never cross a quarter boundary.

    Returns list of (t0, nt, quarter, last_of_quarter)."""
    chunks = []
    for q in range(NW):
        t = QT0[q]
        end = QT0[q] + Q_TILES[q]
        while t < end:
            nt = min(4, end - t)
            chunks.append((t, nt, q, t + nt == end))
            t += nt
    return chunks


def _quarter_pairs():
    """Per quarter: list of (t, is_pair) epilogue units (pairs + 1 single)."""
    units = []
    for q in range(NW):
        t = QT0[q]
        end = QT0[q] + Q_TILES[q]
        while t < end:
            if t + 1 < end:
                units.append((t, True))
                t += 2
            else:
                units.append((t, False))
                t += 1
    return units


def _build_program(sched, alpha1, alpha2):
    nc = bacc.Bacc("TRN2", target_bir_lowering=False, debug=False,
                   num_devices=NCORES, num_swdge_queues=4,
                   dynamic_dma_scratch_size=32768)
    totblk = sched["totblk"]
    blocks = sched["blocks"]
    calls = sched["calls"]

    def inp(name, shape, dt=F32):
        return nc.declare_dram_parameter(name, shape, dt, isOutput=False)

    xT = inp("xT", [CIN, SHARD_PAD], BF)
    idxd = inp("idx", [128, totblk * 8], I16)
    pcold = inp("pcol", [128, totblk], BF)
    dinv = inp("dinv", [128, NT])
    dinv2 = inp("dinv2", [128, NT])
    batchf = inp("batchf", [128, NT])
    iotab = inp("iotab", [128, 128], BF)
    iotaf = inp("iotaf", [128, 256])
    ident = inp("ident", [128, 128])
    Wcat = inp("Wcat", [CIN, 128], BF)           # [W1 | Wr1]
    Wr2x2 = inp("Wr2x2", [128, 2 * H2], BF)      # blockdiag(Wr2, Wr2)
    W2x2 = inp("W2x2", [128, 2 * H2], BF)        # blockdiag(W2, W2)
    b1b2 = inp("b1b2", [128, 128])               # [b1 | b1] broadcast rows
    br1b = inp("br1b", [128, H1])
    br2x2 = inp("br2x2", [128, 2 * H2])
    b2x2 = inp("b2x2", [128, 2 * H2])
    Wf1t = inp("Wf1t", [16, 80])
    Wf2 = inp("Wf2", [80, NCLS])
    bf2r = inp("bf2r", [1, NCLS])
    mcin = inp("mcin", [1, 80])
    out = nc.declare_dram_parameter("out", [G, NCLS], F32, isOutput=True)

    SILU = mybir.ActivationFunctionType.Silu
    ISEQ = mybir.AluOpType.is_equal
    ADD = mybir.AluOpType.add

    with tile.TileContext(nc) as tc:
        with tc.tile_pool(name="const", bufs=1) as constp, \
             tc.tile_pool(name="store", bufs=1) as storep, \
             tc.tile_pool(name="msg", bufs=10) as msgp, \
             tc.tile_pool(name="sp", bufs=12) as sp, \
             tc.tile_pool(name="stg", bufs=3) as stgp, \
             tc.tile_pool(name="ep", bufs=8) as ep, \
             tc.tile_pool(name="dram", bufs=1, space="DRAM") as dram, \
             tc.tile_pool(name="ps_tp", bufs=1, space="PSUM") as ps_tp, \
             tc.tile_pool(name="ps_mm", bufs=2, space="PSUM") as ps_mm, \
             tc.tile_pool(name="ps_agg", bufs=4, space="PSUM") as ps_agg, \
             tc.tile_pool(name="ps_pool", bufs=1, space="PSUM") as ps_pool:

            nc.gpsimd.load_library(mlp_lib)

            # ---- resident constants / state ----
            def ld(ap_src, shape, dt=F32, tag=None):
                t = constp.tile(shape, dt, tag=tag or ap_src.tensor.name)
                nc.sync.dma_start(out=t[:], in_=ap_src)
                return t

            dinv_sb = ld(dinv[:], [128, NT])
            dinv2_sb = ld(dinv2[:], [128, NT])
            batch_sb = ld(batchf[:], [128, NT])
            iotab_sb = ld(iotab[:], [128, 128], BF)
            iotaf_sb = ld(iotaf[:], [128, 256])
            ident_sb = ld(ident[:], [128, 128])
            Wcat_sb = ld(Wcat[:], [CIN, 128], BF)
            Wr2x2_sb = ld(Wr2x2[:], [128, 2 * H2], BF)
            W2x2_sb = ld(W2x2[:], [128, 2 * H2], BF)
            b1b2_sb = ld(b1b2[:], [128, 128])
            br1_sb = ld(br1b[:], [128, H1])
            br2x2_sb = ld(br2x2[:], [128, 2 * H2])
            b2x2_sb = ld(b2x2[:], [128, 2 * H2])
            Wf1t_sb = ld(Wf1t[:], [16, 80])
            Wf2_sb = ld(Wf2[:], [80, NCLS])
            bf2_sb = ld(bf2r[:], [1, NCLS])
            mc_sb = ld(mcin[:], [1, 80])
            xT_sb = ld(xT[:], [CIN, SHARD_PAD], BF)
            idx_sb = ld(idxd[:], [128, totblk * 8], I16)
            pcol_sb = ld(pcold[:], [128, totblk], BF)
            ones1 = constp.tile([1, 128], F32, tag="ones1")
            nc.vector.memset(ones1[:], 1.0)

            accs = storep.tile([128, NT * H1], F32, tag="accs")
            r1_store = storep.tile([128, NT * H1], F32, tag="r1s")
            r2_store = storep.tile([128, NT * H2], F32, tag="r2s")

            hq = [dram.tile([STRIPE[q], 128], BF, tag=f"hq{q}",
                            name=f"hq{q}") for q in range(NW)]
            tbl = [dram.tile([WROWS[w], 128], BF, tag=f"tbl{w}",
                             name=f"tbl{w}") for w in range(NW)]
            pool_in = dram.tile([G, 17], F32, tag="pin")
            pool_out = dram.tile([G, 17], F32, tag="pout")

            chunks = _quarter_chunks()
            pairs = _quarter_pairs()
            # map: tile index -> epilogue unit ending at that tile
            unit_end = {}
            for (t, is_pair) in pairs:
                unit_end[t + 1 if is_pair else t] = (t, is_pair)

            def emit_allgather(q, hsrc):
                nc.gpsimd.collective_compute(
                    "AllGather", mybir.AluOpType.bypass,
                    replica_groups=[list(range(NCORES))],
                    ins=[hsrc.opt()], outs=[tbl[q].opt()])

            # ---------------- stage 0: h1 shard + r1 + acc init ----------
            for (t0, ntc, q, qlast) in chunks:
                stg = stgp.tile([128, 4 * 128], BF, tag="h1stg")
                for a in range(ntc):
                    t = t0 + a
                    mm = ps_mm.tile([128, 128], F32, tag="mm")
                    nc.tensor.matmul(out=mm[:],
                                     lhsT=xT_sb[:, t * 128:(t + 1) * 128],
                                     rhs=Wcat_sb[:], start=True, stop=True)
                    # acc init: (x@W1) * dinv^2   (self-loop contribution)
                    nc.vector.tensor_scalar_mul(
                        out=accs[:, t * H1:(t + 1) * H1], in0=mm[:, 0:H1],
                        scalar1=dinv2_sb[:, t:t + 1])
                    # table row content: (x@W1) * dinv
                    nc.vector.memset(stg[:, a * 128 + H1:(a + 1) * 128], 0.0)
                    nc.vector.tensor_scalar_mul(
                        out=stg[:, a * 128:a * 128 + H1], in0=mm[:, 0:H1],
                        scalar1=dinv_sb[:, t:t + 1])
                    # r1 = silu(x@Wr1 + br1) * alpha1
                    r1t = ep.tile([128, H1], F32, tag="r1t")
                    nc.vector.tensor_add(out=r1t[:], in0=mm[:, H1:128],
                                         in1=br1_sb[:])
                    nc.scalar.activation(out=r1t[:], in_=r1t[:], func=SILU)
                    nc.vector.tensor_scalar_mul(
                        out=r1_store[:, t * H1:(t + 1) * H1], in0=r1t[:],
                        scalar1=float(alpha1))
                r0 = (t0 - QT0[q]) * 128
                nc.scalar.dma_start(
                    out=hq[q][r0:r0 + ntc * 128, :].rearrange(
                        "(a p) c -> p a c", p=128),
                    in_=stg[:, :ntc * 128])
                if qlast:
                    emit_allgather(q, hq[q])

            # ---------------- gather + aggregate (both layers) -----------
            qctr = [0]

            def run_layer(layer):
                agg_of = {}

                def do_epilogue(t, is_pair):
                    if layer == 1:
                        epilogue1(t, is_pair)
                    else:
                        epilogue2(t, is_pair)

                for (blk0, nb, w) in calls:
                    mt = msgp.tile([128, MAXBLK * 128], BF, tag="mt")
                    nc.gpsimd.dma_gather(
                        mt[:, :nb * 128].rearrange("p (b d) -> p b d", d=128),
                        tbl[w][:, :],
                        idx_sb[:, blk0 * 8:(blk0 + nb) * 8],
                        nb * 128, nb * 128, 128,
                        queue_num=qctr[0] % 4,
                    )
                    qctr[0] += 1
                    for j in range(nb):
                        bi = blk0 + j
                        bw, t, first, last = blocks[bi]
                        st = sp.tile([128, 128], BF, tag="st")
                        nc.vector.tensor_tensor(
                            out=st[:],
                            in0=pcol_sb[:, bi:bi + 1].to_broadcast([128, 128]),
                            in1=iotab_sb[:], op=ISEQ)
                        if first:
                            agg_of[t] = ps_agg.tile([128, H1], F32, tag="agg",
                                                    name=f"agg{layer}_{w}_{t}")
                        nc.tensor.matmul(out=agg_of[t][:], lhsT=st[:],
                                         rhs=mt[:, j * 128:j * 128 + H1],
                                         start=first, stop=last)
                        if last:
                            nc.vector.tensor_tensor(
                                out=accs[:, t * H1:(t + 1) * H1],
                                in0=accs[:, t * H1:(t + 1) * H1],
                                in1=agg_of[t][:], op=ADD)
                            del agg_of[t]
                            if w == NW - 1 and t in unit_end:
                                do_epilogue(*unit_end[t])

            # ---- layer 1 epilogue: h = silu(acc*dinv + b1) + r1;
            #      h2 table row = h*dinv; acc := h*dinv^2; r2 = silu(h@Wr2+br2)*a2
            h2stash = [None]   # staging tile + filled tile count

            def flush_h2(q):
                stg, tlist = h2stash[0]
                t0 = tlist[0]
                r0 = (t0 - QT0[q]) * 128
                nc.scalar.dma_start(
                    out=hq[q][r0:r0 + len(tlist) * 128, :].rearrange(
                        "(a p) c -> p a c", p=128),
                    in_=stg[:, :len(tlist) * 128])
                h2stash[0] = None

            def epilogue1(t, is_pair):
                ntl = 2 if is_pair else 1
                hpair = ep.tile([128, 128], F32, tag="hpair")
                if ntl == 1:
                    nc.vector.memset(hpair[:, H1:128], 0.0)
                for a in range(ntl):
                    tt = t + a
                    # acc*dinv + b1
                    nc.vector.tensor_scalar_mul(
                        out=hpair[:, a * H1:(a + 1) * H1],
                        in0=accs[:, tt * H1:(tt + 1) * H1],
                        scalar1=dinv_sb[:, tt:tt + 1])
                nc.vector.tensor_add(out=hpair[:, :ntl * H1],
                                     in0=hpair[:, :ntl * H1],
                                     in1=b1b2_sb[:, :ntl * H1])
                nc.scalar.activation(out=hpair[:, :ntl * H1],
                                     in_=hpair[:, :ntl * H1], func=SILU)
                nc.vector.tensor_add(out=hpair[:, :ntl * H1],
                                     in0=hpair[:, :ntl * H1],
                                     in1=r1_store[:, t * H1:(t + ntl) * H1])
                q = next(qq for qq in range(NW)
                         if QT0[qq] <= t < QT0[qq] + Q_TILES[qq])
                if h2stash[0] is None:
                    stgt = stgp.tile([128, 4 * 128], BF, tag="h2stg")
                    h2stash[0] = (stgt, [])
                stgt, tlist = h2stash[0]
                for a in range(ntl):
                    tt = t + a
                    pos = len(tlist)
                    nc.vector.memset(stgt[:, pos * 128 + H1:(pos + 1) * 128],
                                     0.0)
                    nc.vector.tensor_scalar_mul(
                        out=stgt[:, pos * 128:pos * 128 + H1],
                        in0=hpair[:, a * H1:(a + 1) * H1],
                        scalar1=dinv_sb[:, tt:tt + 1])
                    # acc := h * dinv^2 for layer-2 self-loop
                    nc.vector.tensor_scalar_mul(
                        out=accs[:, tt * H1:(tt + 1) * H1],
                        in0=hpair[:, a * H1:(a + 1) * H1],
                        scalar1=dinv2_sb[:, tt:tt + 1])
                    tlist.append(tt)
                last_of_q = (t + ntl == QT0[q] + Q_TILES[q])
                if len(tlist) == 4 or last_of_q:
                    flush_h2(q)
                    if last_of_q:
                        emit_allgather(q, hq[q])
                # r2 = silu(h @ Wr2 + br2) * alpha2
                tp = ps_tp.tile([128, 128], F32, tag="tp")
                nc.tensor.transpose(out=tp[:, :], in_=hpair[:],
                                    identity=ident_sb[:])
                hT = ep.tile([128, 128], BF, tag="hT")
                nc.scalar.copy(out=hT[:], in_=tp[:])
                r2ps = ps_mm.tile([128, 128], F32, tag="mm")
                nc.tensor.matmul(out=r2ps[:, 0:2 * H2], lhsT=hT[:],
                                 rhs=Wr2x2_sb[:], start=True, stop=True)
                r2t = ep.tile([128, 2 * H2], F32, tag="r2t")
                nc.vector.tensor_add(out=r2t[:, :ntl * H2], in0=r2ps[:, :ntl * H2],
                                     in1=br2x2_sb[:, :ntl * H2])
                nc.scalar.activation(out=r2t[:, :ntl * H2],
                                     in_=r2t[:, :ntl * H2], func=SILU)
                nc.vector.tensor_scalar_mul(
                    out=r2_store[:, t * H2:(t + ntl) * H2],
                    in0=r2t[:, :ntl * H2], scalar1=float(alpha2))

            # ---- layer 2 epilogue: z = (acc*dinv)@W2 + b2 + r2; pooling
            pool_ps = ps_pool.tile([128, 34], F32, tag="pool")
            tcount = [0]

            def epilogue2(t, is_pair):
                ntl = 2 if is_pair else 1
                zpre = ep.tile([128, 128], F32, tag="zpre")
                if ntl == 1:
                    nc.vector.memset(zpre[:, H1:128], 0.0)
                for a in range(ntl):
                    tt = t + a
                    nc.vector.tensor_scalar_mul(
                        out=zpre[:, a * H1:(a + 1) * H1],
                        in0=accs[:, tt * H1:(tt + 1) * H1],
                        scalar1=dinv_sb[:, tt:tt + 1])
                tp = ps_tp.tile([128, 128], F32, tag="tp")
                nc.tensor.transpose(out=tp[:, :], in_=zpre[:],
                                    identity=ident_sb[:])
                zT = ep.tile([128, 128], BF, tag="zT")
                nc.scalar.copy(out=zT[:], in_=tp[:])
                zps = ps_mm.tile([128, 128], F32, tag="mm")
                nc.tensor.matmul(out=zps[:, 0:2 * H2], lhsT=zT[:],
                                 rhs=W2x2_sb[:], start=True, stop=True)
                zfin = ep.tile([128, 2 * H2], F32, tag="zfin")
                nc.vector.tensor_add(out=zfin[:, :ntl * H2],
                                     in0=zps[:, :ntl * H2],
                                     in1=b2x2_sb[:, :ntl * H2])
                nc.vector.tensor_add(out=zfin[:, :ntl * H2],
                                     in0=zfin[:, :ntl * H2],
                                     in1=r2_store[:, t * H2:(t + ntl) * H2])
                for a in range(ntl):
                    tt = t + a
                    zext = ep.tile([128, 17], F32, tag="zext")
                    nc.vector.tensor_copy(out=zext[:, 0:H2],
                                          in_=zfin[:, a * H2:(a + 1) * H2])
                    nc.vector.memset(zext[:, H2:], 1.0)
                    k = tcount[0]
                    s0 = ep.tile([128, 128], F32, tag="s0")
                    nc.vector.tensor_tensor(
                        out=s0[:],
                        in0=batch_sb[:, tt:tt + 1].to_broadcast([128, 128]),
                        in1=iotaf_sb[:, 0:128], op=ISEQ)
                    nc.tensor.matmul(out=pool_ps[:, 0:17], lhsT=s0[:],
                                     rhs=zext[:], start=False,
                                     stop=(k == NT - 1),
                                     skip_group_check=True)
                    s1 = ep.tile([128, 128], F32, tag="s1")
                    nc.vector.tensor_tensor(
                        out=s1[:],
                        in0=batch_sb[:, tt:tt + 1].to_broadcast([128, 128]),
                        in1=iotaf_sb[:, 128:256], op=ISEQ)
                    nc.tensor.matmul(out=pool_ps[:, 17:34], lhsT=s1[:],
                                     rhs=zext[:], start=False,
                                     stop=(k == NT - 1),
                                     skip_group_check=True)
                    tcount[0] += 1

            run_layer(1)
            nc.vector.memset(pool_ps[:], 0.0)
            run_layer(2)

            psums = ep.tile([128, 34], F32, tag="psums")
            nc.vector.tensor_copy(out=psums[:], in_=pool_ps[:])
            nc.sync.dma_start(out=pool_in[0:128, :], in_=psums[:, 0:17])
            nc.sync.dma_start(out=pool_in[128:256, :], in_=psums[:, 17:34])

            nc.gpsimd.collective_compute(
                "AllReduce", mybir.AluOpType.add,
                replica_groups=[list(range(NCORES))],
                ins=[pool_in.opt()], outs=[pool_out.opt()])

            # ---------------- classifier head (two graph windows) ----------
            for wdw in range(2):
                sums = ep.tile([128, 17], F32, tag="hsum")
                nc.sync.dma_start(out=sums[:],
                                  in_=pool_out[wdw * 128:(wdw + 1) * 128, :])
                cnt = ep.tile([128, 1], F32, tag="hcnt")
                nc.vector.tensor_scalar_max(out=cnt[:], in0=sums[:, 16:17],
                                            scalar1=1.0)
                rec = ep.tile([128, 1], F32, tag="hrec")
                nc.vector.reciprocal(out=rec[:], in_=cnt[:])
                ge = ep.tile([128, 16], F32, tag="hge")
                nc.vector.tensor_scalar_mul(out=ge[:], in0=sums[:, :16],
                                            scalar1=rec[:])
                geT_ps = ps_tp.tile([128, 128], F32, tag="tp")
                nc.tensor.transpose(out=geT_ps[:16, :], in_=ge[:],
                                    identity=ident_sb[:])
                geT = ep.tile([16, 128], F32, tag="hget")
                nc.vector.tensor_copy(out=geT[:], in_=geT_ps[:16, :])
                u_ps = ps_mm.tile([128, 128], F32, tag="mm")
                nc.tensor.matmul(out=u_ps[:, 0:80], lhsT=geT[:], rhs=Wf1t_sb[:],
                                 start=True, stop=False)
                nc.tensor.matmul(out=u_ps[:, 0:80], lhsT=ones1[:], rhs=mc_sb[:],
                                 start=False, stop=True)
                u = ep.tile([128, 80], F32, tag="hu")
                nc.scalar.activation(out=u[:], in_=u_ps[:, 0:80], func=SILU)
                uT_ps = ps_tp.tile([128, 128], F32, tag="tp")
                nc.tensor.transpose(out=uT_ps[:80, :], in_=u[:],
                                    identity=ident_sb[:])
                uT = ep.tile([80, 128], F32, tag="hut")
                nc.vector.tensor_copy(out=uT[:], in_=uT_ps[:80, :])
                o_ps = ps_mm.tile([128, 128], F32, tag="mm")
                nc.tensor.matmul(out=o_ps[:, 0:NCLS], lhsT=uT[:], rhs=Wf2_sb[:],
                                 start=True, stop=False)
                nc.tensor.matmul(out=o_ps[:, 0:NCLS], lhsT=ones1[:], rhs=bf2_sb[:],
                                 start=False, stop=True)
                o = ep.tile([128, NCLS], F32, tag="ho")
                nc.vector.tensor_copy(out=o[:], in_=o_ps[:, 0:NCLS])
                nc.sync.dma_start(out=out[wdw * 128:(wdw + 1) * 128, :],
                                  in_=o[:])

    nc.compile()
    return nc


def _host_metrics_contrib(tolerance, cost, time, quantity,
                          mW1, mb1, mW2, mb2, Wf1, bf1):
    silu = lambda v: v / (1.0 + np.exp(-v))
    m = np.stack([np.asarray(v, np.float32).reshape(1, 1) for v in
                  (tolerance, cost, time, quantity)])         # [4,1,1]
    e = silu(np.einsum('gij,gjk->gik', m, np.asarray(mW1, np.float32))
             + np.asarray(mb1, np.float32)[:, None, :])
    e = (np.einsum('gij,gjk->gik', e, np.asarray(mW2, np.float32))
         + np.asarray(mb2, np.float32)[:, None, :])           # [4,1,16]
    metvec = e.transpose(1, 0, 2).reshape(1, 64)
    mc = metvec @ np.asarray(Wf1, np.float32)[16:, :] + np.asarray(bf1, np.float32)[None, :]
    return mc.astype(np.float32)


def _blockdiag2(W):
    """[64,16] -> [128,32] blockdiag bf16."""
    out = np.zeros((128, 32), np.float32)
    out[:64, :16] = W
    out[64:, 16:] = W
    return out.astype(BF16)


def kernel(x, edge_index, batch, tolerance, cost, time, quantity,
           W1, b1, W2, b2, Wr1, br1, Wr2, br2, alpha1, alpha2,
           mW1, mb1, mW2, mb2, Wf1, bf1, Wf2, bf2):
    x = np.asarray(x, np.float32)
    src = np.asarray(edge_index[0], np.int64)
    dst = np.asarray(edge_index[1], np.int64)
    batch = np.asarray(batch, np.int64)

    deg = 1.0 + np.bincount(dst, minlength=N).astype(np.float32)
    dinv_full = 1.0 / np.sqrt(deg)

    sched, per_core = _build_structure(src, dst)
    nc = _build_program(sched, float(alpha1), float(alpha2))

    W1f = np.asarray(W1, np.float32)
    Wr1f = np.asarray(Wr1, np.float32)
    Wcat = np.concatenate([W1f, Wr1f], axis=1).astype(BF16)   # [128,128]
    b1f = np.asarray(b1, np.float32)
    common = {
        "iotab": np.tile(np.arange(128, dtype=np.float32), (128, 1)).astype(BF16),
        "iotaf": np.tile(np.arange(256, dtype=np.float32), (128, 1)),
        "ident": np.eye(128, dtype=np.float32),
        "Wcat": Wcat,
        "Wr2x2": _blockdiag2(np.asarray(Wr2, np.float32)),
        "W2x2": _blockdiag2(np.asarray(W2, np.float32)),
        "b1b2": np.tile(np.concatenate([b1f, b1f])[None, :], (128, 1)),
        "br1b": np.tile(np.asarray(br1, np.float32), (128, 1)),
        "br2x2": np.tile(np.concatenate([np.asarray(br2, np.float32)] * 2)[None, :], (128, 1)),
        "b2x2": np.tile(np.concatenate([np.asarray(b2, np.float32)] * 2)[None, :], (128, 1)),
        "Wf1t": np.asarray(Wf1[:16, :], np.float32),
        "Wf2": np.asarray(Wf2, np.float32),
        "bf2r": np.asarray(bf2, np.float32)[None, :],
        "mcin": _host_metrics_contrib(tolerance, cost, time, quantity,
                                      mW1, mb1, mW2, mb2, Wf1, bf1),
    }

    in_maps = []
    for c in range(NCORES):
        lo, hi = c * SHARD, (c + 1) * SHARD
        xs = np.zeros((SHARD_PAD, CIN), np.float32)
        xs[:SHARD] = x[lo:hi]
        dv = np.zeros(SHARD_PAD, np.float32)
        dv[:SHARD] = dinv_full[lo:hi]
        bf_loc = np.full(SHARD_PAD, -1.0, np.float32)
        bf_loc[:SHARD] = batch[lo:hi].astype(np.float32)
        m = dict(common)
        m["xT"] = np.ascontiguousarray(xs.T).astype(BF16)
        m["dinv"] = dv.reshape(NT, 128).T.copy()
        m["dinv2"] = (dv * dv).reshape(NT, 128).T.copy()
        m["batchf"] = bf_loc.reshape(NT, 128).T.copy()
        m["idx"] = per_core[c]["idx"]
        m["pcol"] = per_core[c]["pcol"]
        in_maps.append(m)

    res = run_bass_kernel_spmd(nc, in_maps, list(range(NCORES)))
    kernel._last = (nc, in_maps)   # for external profiling harnesses
    kernel._res = res
    return np.asarray(res.results[0]["out"], np.float32)


# revision 11
# speedup vs baseline: 1.0630x; 1.0255x over previous
"""GCN classifier with metrics — TRN2 Bass kernel (8 NeuronCores, SPMD).

Strategy (v3):
  - Nodes partitioned contiguously across 8 cores (12500/core, padded to
    12544 = 98 tiles). Table rows are QUARTER-INTERLEAVED: window w holds
    quarter w of every core's shard, so the AllGather for each layer splits
    into 4 sub-AllGathers that overlap with gather/aggregate compute.
  - Per layer: h_scaled = (x @ W) * dinv[src] per-shard (bf16), 4
    sub-AllGathers build the replicated table (4 DRAM window tensors,
    [WROWS, 128] bf16 rows: 64 feats + 64 zero pad = 256B).
  - Each window (<=6.6 MB) is bulk-loaded into SBUF; edge gathers then run
    SBUF->SBUF via dma_gather(transpose=True) — random 256B HBM reads cost
    ~70ns/descriptor (latency-bound), SBUF reads don't. Token layout:
    window row r lives at partition r//tpp, byte offset (r%tpp)*256, so the
    bulk load is 128 contiguous descriptors and idx = (r%tpp)*128 + r//tpp.
  - The transposed gather yields msgT [feat, slot] blocks; each 128-slot
    block is PE-transposed back (K=64) and scalar-copied to SBUF, then
    contracted with an ON-CHIP one-hot S [slot, dst] (vector is_equal on a
    resident per-block dst-partition column) into a per-run PSUM
    accumulator, added into a per-tile SBUF f32 accumulator. Self-loops are
    folded into the accumulator init acc = (x@W) * dinv^2.
  - global_mean_pool: indicator matmul per tile into a PSUM accumulator over
    two 128-graph windows, AllReduce [256,17] across cores, head computed
    redundantly on every core.
"""
import sys
import numpy as np

sys.path.insert(0, "/opt/trn_rl_repo")

import ml_dtypes
import concourse.bass as bass
import concourse.bacc as bacc
import concourse.mybir as mybir
import concourse.tile as tile
from concourse.bass_utils import run_bass_kernel_spmd
from concourse.library_config import mlp as mlp_lib

BF16 = ml_dtypes.bfloat16

N = 100_000
E = 1_600_000
G = 256
CIN = 128
NCLS = 10
NCORES = 8
SHARD = 12_500
SHARD_PAD = 12_544          # 98 * 128
NT = 98                     # tiles per core
H1 = 64
H2 = 16
MAXBLK = 6                  # blocks/call (768 idx; SBUF-transpose gather breaks at 1024)
NW = 4                      # source windows
Q_TILES = [25, 25, 25, 23]          # tiles per local quarter
STRIPE = [3200, 3200, 3200, 2944]   # rows per (core, quarter)
WROWS = [25600, 25600, 25600, 23552]
TPP = [200, 200, 200, 184]          # window rows per partition
QT0 = [0, 25, 50, 75]               # first tile of each quarter

F32 = mybir.dt.float32
BF = mybir.dt.bfloat16
I16 = mybir.dt.int16


def _wrap_idx(idx):
    """[n] int16 (n % 128 == 0) -> [128, n//16] wrapped + replicated layout."""
    n = len(idx)
    w = idx.reshape(n // 16, 16).T.astype(np.int16)   # [16, n/16]
    return np.tile(w, (8, 1))


def _build_structure(src, dst):
    """Slot structure shared by both layers.

    Returns (sched, per_core):
      sched: blocks [(w, t, first, last)], calls [(blk0, nb, w)], totblk
      per_core: list of dicts with idx [128, totblk*8] int16,
                pcol [128, totblk] bf16
    """
    g = np.arange(N, dtype=np.int64)
    c_of = g // SHARD
    i_of = g % SHARD
    q_of = np.minimum(i_of // 3200, 3)
    o_of = i_of - q_of * 3200
    stripe = np.array(STRIPE, dtype=np.int64)
    node_win = q_of.astype(np.int64)
    node_rel = o_of + c_of * stripe[q_of]     # row within window

    ecore = dst // SHARD
    order0 = np.argsort(ecore, kind="stable")
    src_o, dst_o = src[order0], dst[order0]
    cbounds = np.searchsorted(ecore[order0], np.arange(NCORES + 1))

    counts = np.zeros((NCORES, NW, NT), np.int64)
    percore_sorted = []
    for c in range(NCORES):
        s_c = src_o[cbounds[c]:cbounds[c + 1]]
        d_c = dst_o[cbounds[c]:cbounds[c + 1]]
        dloc = d_c - c * SHARD
        t = dloc // 128
        p = dloc % 128
        w = node_win[s_c]
        rel = node_rel[s_c]
        o2 = np.lexsort((rel, t, w))
        t, p, w, rel = t[o2], p[o2], w[o2], rel[o2]
        key = w * NT + t
        counts[c] = np.bincount(key, minlength=NW * NT).reshape(NW, NT)
        percore_sorted.append((key, rel, p))

    maxc = counts.max(axis=0)                         # [NW, NT]
    assert (maxc > 0).all()
    nblk = (maxc + 127) // 128                        # [NW, NT]

    blocks = []          # (w, t, first_of_run, last_of_run)
    run_blk_start = np.zeros(NW * NT, np.int64)
    acc_b = 0
    for w in range(NW):
        for t in range(NT):
            b = int(nblk[w, t])
            run_blk_start[w * NT + t] = acc_b
            for j in range(b):
                blocks.append((w, t, j == 0, j == b - 1))
            acc_b += b
    totblk = acc_b

    calls = []
    i = 0
    while i < totblk:
        w = blocks[i][0]
        nb = 1
        while nb < MAXBLK and i + nb < totblk and blocks[i + nb][0] == w:
            nb += 1
        calls.append((i, nb, w))
        i += nb

    tpp = np.array(TPP, dtype=np.int64)
    per_core = []
    for c in range(NCORES):
        key, rel, p = percore_sorted[c]
        w_of = key // NT
        tok = (rel % tpp[w_of]) * 128 + rel // tpp[w_of]   # SBUF token id
        idxs = np.zeros(totblk * 128, np.int16)
        pc = np.full((128, totblk), 999.0, np.float32)
        grp_first = np.searchsorted(key, np.arange(NW * NT), side="left")
        ranks = np.arange(len(key)) - grp_first[key]
        slot = run_blk_start[key] * 128 + ranks
        idxs[slot] = tok.astype(np.int16)
        pc[slot % 128, slot // 128] = p
        per_core.append({"idx": _wrap_idx(idxs),
                         "pcol": pc.astype(BF16)})

    sched = {"blocks": blocks, "calls": calls, "totblk": totblk,
             "nblk": nblk}
    return sched, per_core


def _quarter_chunks():
    """Tile chunks (<=4 tiles) that never cross a quarter boundary.

    Returns list of (t0, nt, quarter, last_of_quarter)."""
    chunks = []
    for q in range(NW):
        t = QT0[q]
        end = QT0[q] + Q_TILES[q]
        while t < end:
            nt = min(4, end - t)
            chunks.append((t, nt, q, t + nt == end))
            t += nt
    return chunks


def _quarter_pairs():
    """Per quarter: list of (t, is_pair) epilogue units (pairs + 1 single)."""
    units = []
    for q in range(NW):
        t = QT0[q]
        end = QT0[q] + Q_TILES[q]
        while t < end:
            if t + 1 < end:
                units.append((t, True))
                t += 2
            else:
                units.append((t, False))
                t += 1
    return units


def _build_program(sched, alpha1, alpha2):
    nc = bacc.Bacc("TRN2", target_bir_lowering=False, debug=False,
                   num_devices=NCORES, num_swdge_queues=4,
                   dynamic_dma_scratch_size=16384)
    totblk = sched["totblk"]
    blocks = sched["blocks"]
    calls = sched["calls"]

    def inp(name, shape, dt=F32):
        return nc.declare_dram_parameter(name, shape, dt, isOutput=False)

    xT = inp("xT", [CIN, SHARD_PAD], BF)
    idxd = inp("idx", [128, totblk * 8], I16)
    pcold = inp("pcol", [128, totblk], BF)
    dinv = inp("dinv", [128, NT])
    dinv2 = inp("dinv2", [128, NT])
    batchf = inp("batchf", [128, NT])
    iotab = inp("iotab", [128, 128], BF)
    iotaf = inp("iotaf", [128, 256])
    ident = inp("ident", [128, 128])
    identb = inp("identb", [128, 128], BF)
    Wcat = inp("Wcat", [CIN, 128], BF)           # [W1 | Wr1]
    Wr2x2 = inp("Wr2x2", [128, 2 * H2], BF)      # blockdiag(Wr2, Wr2)
    W2x2 = inp("W2x2", [128, 2 * H2], BF)        # blockdiag(W2, W2)
    b1b2 = inp("b1b2", [128, 128])               # [b1 | b1] broadcast rows
    br1b = inp("br1b", [128, H1])
    br2x2 = inp("br2x2", [128, 2 * H2])
    b2x2 = inp("b2x2", [128, 2 * H2])
    Wf1t = inp("Wf1t", [16, 80])
    Wf2 = inp("Wf2", [80, NCLS])
    bf2r = inp("bf2r", [1, NCLS])
    mcin = inp("mcin", [1, 80])
    out = nc.declare_dram_parameter("out", [G, NCLS], F32, isOutput=True)

    SILU = mybir.ActivationFunctionType.Silu
    ISEQ = mybir.AluOpType.is_equal
    ADD = mybir.AluOpType.add

    with tile.TileContext(nc) as tc:
        with tc.tile_pool(name="const", bufs=1) as constp, \
             tc.tile_pool(name="store", bufs=1) as storep, \
             tc.tile_pool(name="dram", bufs=1, space="DRAM") as dram, \
             tc.tile_pool(name="ps_tp", bufs=1, space="PSUM") as ps_tp, \
             tc.tile_pool(name="ps_tpb", bufs=2, space="PSUM") as ps_tpb, \
             tc.tile_pool(name="ps_mm", bufs=2, space="PSUM") as ps_mm, \
             tc.tile_pool(name="ps_agg", bufs=2, space="PSUM") as ps_agg, \
             tc.tile_pool(name="ps_pool", bufs=1, space="PSUM") as ps_pool:

            nc.gpsimd.load_library(mlp_lib)

            # ---- resident constants / state ----
            def ld(ap_src, shape, dt=F32, tag=None):
                t = constp.tile(shape, dt, tag=tag or ap_src.tensor.name)
                nc.sync.dma_start(out=t[:], in_=ap_src)
                return t

            dinv_sb = ld(dinv[:], [128, NT])
            dinv2_sb = ld(dinv2[:], [128, NT])
            batch_sb = ld(batchf[:], [128, NT])
            iotab_sb = ld(iotab[:], [128, 128], BF)
            iotaf_sb = ld(iotaf[:], [128, 256])
            ident_sb = ld(ident[:], [128, 128])
            identb_sb = ld(identb[:], [128, 128], BF)
            Wcat_sb = ld(Wcat[:], [CIN, 128], BF)
            Wr2x2_sb = ld(Wr2x2[:], [128, 2 * H2], BF)
            W2x2_sb = ld(W2x2[:], [128, 2 * H2], BF)
            b1b2_sb = ld(b1b2[:], [128, 128])
            br1_sb = ld(br1b[:], [128, H1])
            br2x2_sb = ld(br2x2[:], [128, 2 * H2])
            b2x2_sb = ld(b2x2[:], [128, 2 * H2])
            Wf1t_sb = ld(Wf1t[:], [16, 80])
            Wf2_sb = ld(Wf2[:], [80, NCLS])
            bf2_sb = ld(bf2r[:], [1, NCLS])
            mc_sb = ld(mcin[:], [1, 80])
            pcol_sb = ld(pcold[:], [128, totblk], BF)
            ones1 = constp.tile([1, 128], F32, tag="ones1")
            nc.vector.memset(ones1[:], 1.0)

            accs = storep.tile([128, NT * H1], F32, tag="accs")
            r1_store = storep.tile([128, NT * H1], F32, tag="r1s")
            r2_store = storep.tile([128, NT * H2], F32, tag="r2s")

            hq = [dram.tile([STRIPE[q], 128], BF, tag=f"hq{q}",
                            name=f"hq{q}") for q in range(NW)]
            tbl = [dram.tile([WROWS[w], 128], BF, tag=f"tbl{w}",
                             name=f"tbl{w}") for w in range(NW)]
            pool_in = dram.tile([G, 17], F32, tag="pin")
            pool_out = dram.tile([G, 17], F32, tag="pout")

            chunks = _quarter_chunks()
            pairs = _quarter_pairs()
            unit_end = {}
            for (t, is_pair) in pairs:
                unit_end[t + 1 if is_pair else t] = (t, is_pair)

            def emit_allgather(q, hsrc):
                nc.gpsimd.collective_compute(
                    "AllGather", mybir.AluOpType.bypass,
                    replica_groups=[list(range(NCORES))],
                    ins=[hsrc.opt()], outs=[tbl[q].opt()])

            # ---------------- stage 0 (own pool scope, freed after) -------
            with tc.tile_pool(name="xp", bufs=1) as xp, \
                 tc.tile_pool(name="stg0", bufs=3) as stg0p, \
                 tc.tile_pool(name="ep0", bufs=4) as ep0:
                xT_sb = xp.tile([CIN, SHARD_PAD], BF, tag="xT")
                nc.sync.dma_start(out=xT_sb[:], in_=xT[:])
                for (t0, ntc, q, qlast) in chunks:
                    stg = stg0p.tile([128, 4 * 128], BF, tag="h1stg")
                    for a in range(ntc):
                        t = t0 + a
                        mm = ps_mm.tile([128, 128], F32, tag="mm")
                        nc.tensor.matmul(out=mm[:],
                                         lhsT=xT_sb[:, t * 128:(t + 1) * 128],
                                         rhs=Wcat_sb[:], start=True, stop=True)
                        nc.vector.tensor_scalar_mul(
                            out=accs[:, t * H1:(t + 1) * H1], in0=mm[:, 0:H1],
                            scalar1=dinv2_sb[:, t:t + 1])
                        nc.vector.memset(stg[:, a * 128 + H1:(a + 1) * 128], 0.0)
                        nc.vector.tensor_scalar_mul(
                            out=stg[:, a * 128:a * 128 + H1], in0=mm[:, 0:H1],
                            scalar1=dinv_sb[:, t:t + 1])
                        r1t = ep0.tile([128, H1], F32, tag="r1t")
                        nc.vector.tensor_add(out=r1t[:], in0=mm[:, H1:128],
                                             in1=br1_sb[:])
                        nc.scalar.activation(out=r1t[:], in_=r1t[:], func=SILU)
                        nc.vector.tensor_scalar_mul(
                            out=r1_store[:, t * H1:(t + 1) * H1], in0=r1t[:],
                            scalar1=float(alpha1))
                    r0 = (t0 - QT0[q]) * 128
                    nc.scalar.dma_start(
                        out=hq[q][r0:r0 + ntc * 128, :].rearrange(
                            "(a p) c -> p a c", p=128),
                        in_=stg[:, :ntc * 128])
                    if qlast:
                        emit_allgather(q, hq[q])

            # ---------------- gather-phase pools --------------------------
            with tc.tile_pool(name="tblp", bufs=1) as tblp, \
                 tc.tile_pool(name="msg", bufs=8) as msgp, \
                 tc.tile_pool(name="msb", bufs=10) as msb, \
                 tc.tile_pool(name="sp", bufs=12) as sp, \
                 tc.tile_pool(name="idxp", bufs=6) as idxp, \
                 tc.tile_pool(name="stg", bufs=3) as stgp, \
                 tc.tile_pool(name="ep", bufs=8) as ep:

                tbl_sb = tblp.tile([128, 200 * 128], BF, tag="tblsb")

                qctr = [0]

                def run_layer(layer):
                    agg_of = {}

                    def do_epilogue(t, is_pair):
                        if layer == 1:
                            epilogue1(t, is_pair)
                        else:
                            epilogue2(t, is_pair)

                    cur_w = [-1]
                    for (blk0, nb, w) in calls:
                        if w != cur_w[0]:
                            cur_w[0] = w
                            nc.sync.dma_start(
                                out=tbl_sb[:, :TPP[w] * 128],
                                in_=tbl[w][:, :].rearrange(
                                    "(r s) c -> r (s c)", r=128))
                        it = idxp.tile([128, MAXBLK * 8], I16, tag="it")
                        nc.scalar.dma_start(
                            out=it[:, :nb * 8],
                            in_=idxd[:, blk0 * 8:(blk0 + nb) * 8])
                        mtT = msgp.tile([128, MAXBLK * 128], BF, tag="mtT")
                        nc.gpsimd.dma_gather(
                            mtT[:, :nb * 128].rearrange("p (o s) -> p o s", o=1),
                            tbl_sb[:, :TPP[w] * 128],
                            it[:, :nb * 8],
                            nb * 128, nb * 128, 128,
                            transpose=True,
                            queue_num=qctr[0] % 4,
                            sbuf_tokens_per_rank=128,
                            sbuf_free_dim_per_rank=256,
                        )
                        qctr[0] += 1
                        for j in range(nb):
                            bi = blk0 + j
                            bw, t, first, last = blocks[bi]
                            tpb = ps_tpb.tile([128, 128], BF, tag="tpb")
                            nc.tensor.transpose(
                                out=tpb[:, 0:H1],
                                in_=mtT[0:H1, j * 128:(j + 1) * 128],
                                identity=identb_sb[0:H1, 0:H1])
                            msgj = msb.tile([128, H1], BF, tag="msgj")
                            nc.scalar.copy(out=msgj[:], in_=tpb[:, 0:H1])
                            st = sp.tile([128, 128], BF, tag="st")
                            nc.vector.tensor_tensor(
                                out=st[:],
                                in0=pcol_sb[:, bi:bi + 1].to_broadcast([128, 128]),
                                in1=iotab_sb[:], op=ISEQ)
                            if first:
                                agg_of[t] = ps_agg.tile(
                                    [128, H1], F32, tag="agg",
                                    name=f"agg{layer}_{w}_{t}")
                            nc.tensor.matmul(out=agg_of[t][:], lhsT=st[:],
                                             rhs=msgj[:],
                                             start=first, stop=last)
                            if last:
                                nc.vector.tensor_tensor(
                                    out=accs[:, t * H1:(t + 1) * H1],
                                    in0=accs[:, t * H1:(t + 1) * H1],
                                    in1=agg_of[t][:], op=ADD)
                                del agg_of[t]
                                if w == NW - 1 and t in unit_end:
                                    do_epilogue(*unit_end[t])

                # ---- layer 1 epilogue ------------------------------------
                h2stash = [None]

                def flush_h2(q):
                    stg, tlist = h2stash[0]
                    t0 = tlist[0]
                    r0 = (t0 - QT0[q]) * 128
                    nc.scalar.dma_start(
                        out=hq[q][r0:r0 + len(tlist) * 128, :].rearrange(
                            "(a p) c -> p a c", p=128),
                        in_=stg[:, :len(tlist) * 128])
                    h2stash[0] = None

                def epilogue1(t, is_pair):
                    ntl = 2 if is_pair else 1
                    hpair = ep.tile([128, 128], F32, tag="hpair")
                    if ntl == 1:
                        nc.vector.memset(hpair[:, H1:128], 0.0)
                    for a in range(ntl):
                        tt = t + a
                        nc.vector.tensor_scalar_mul(
                            out=hpair[:, a * H1:(a + 1) * H1],
                            in0=accs[:, tt * H1:(tt + 1) * H1],
                            scalar1=dinv_sb[:, tt:tt + 1])
                    nc.vector.tensor_add(out=hpair[:, :ntl * H1],
                                         in0=hpair[:, :ntl * H1],
                                         in1=b1b2_sb[:, :ntl * H1])
                    nc.scalar.activation(out=hpair[:, :ntl * H1],
                                         in_=hpair[:, :ntl * H1], func=SILU)
                    nc.vector.tensor_add(out=hpair[:, :ntl * H1],
                                         in0=hpair[:, :ntl * H1],
                                         in1=r1_store[:, t * H1:(t + ntl) * H1])
                    q = next(qq for qq in range(NW)
                             if QT0[qq] <= t < QT0[qq] + Q_TILES[qq])
                    if h2stash[0] is None:
                        stgt = stgp.tile([128, 4 * 128], BF, tag="h2stg")
                        h2stash[0] = (stgt, [])
                    stgt, tlist = h2stash[0]
                    for a in range(ntl):
                        tt = t + a
                        pos = len(tlist)
                        nc.vector.memset(
                            stgt[:, pos * 128 + H1:(pos + 1) * 128], 0.0)
                        nc.vector.tensor_scalar_mul(
                            out=stgt[:, pos * 128:pos * 128 + H1],
                            in0=hpair[:, a * H1:(a + 1) * H1],
                            scalar1=dinv_sb[:, tt:tt + 1])
                        nc.vector.tensor_scalar_mul(
                            out=accs[:, tt * H1:(tt + 1) * H1],
                            in0=hpair[:, a * H1:(a + 1) * H1],
                            scalar1=dinv2_sb[:, tt:tt + 1])
                        tlist.append(tt)
                    last_of_q = (t + ntl == QT0[q] + Q_TILES[q])
                    if len(tlist) == 4 or last_of_q:
                        flush_h2(q)
                        if last_of_q:
                            emit_allgather(q, hq[q])
                    tp = ps_tp.tile([128, 128], F32, tag="tp")
                    nc.tensor.transpose(out=tp[:, :], in_=hpair[:],
                                        identity=ident_sb[:])
                    hT = ep.tile([128, 128], BF, tag="hT")
                    nc.scalar.copy(out=hT[:], in_=tp[:])
                    r2ps = ps_mm.tile([128, 128], F32, tag="mm")
                    nc.tensor.matmul(out=r2ps[:, 0:2 * H2], lhsT=hT[:],
                                     rhs=Wr2x2_sb[:], start=True, stop=True)
                    r2t = ep.tile([128, 2 * H2], F32, tag="r2t")
                    nc.vector.tensor_add(out=r2t[:, :ntl * H2],
                                         in0=r2ps[:, :ntl * H2],
                                         in1=br2x2_sb[:, :ntl * H2])
                    nc.scalar.activation(out=r2t[:, :ntl * H2],
                                         in_=r2t[:, :ntl * H2], func=SILU)
                    nc.vector.tensor_scalar_mul(
                        out=r2_store[:, t * H2:(t + ntl) * H2],
                        in0=r2t[:, :ntl * H2], scalar1=float(alpha2))

                # ---- layer 2 epilogue + pooling --------------------------
                pool_ps = ps_pool.tile([128, 34], F32, tag="pool")
                tcount = [0]

                def epilogue2(t, is_pair):
                    ntl = 2 if is_pair else 1
                    zpre = ep.tile([128, 128], F32, tag="zpre")
                    if ntl == 1:
                        nc.vector.memset(zpre[:, H1:128], 0.0)
                    for a in range(ntl):
                        tt = t + a
                        nc.vector.tensor_scalar_mul(
                            out=zpre[:, a * H1:(a + 1) * H1],
                            in0=accs[:, tt * H1:(tt + 1) * H1],
                            scalar1=dinv_sb[:, tt:tt + 1])
                    tp = ps_tp.tile([128, 128], F32, tag="tp")
                    nc.tensor.transpose(out=tp[:, :], in_=zpre[:],
                                        identity=ident_sb[:])
                    zT = ep.tile([128, 128], BF, tag="zT")
                    nc.scalar.copy(out=zT[:], in_=tp[:])
                    zps = ps_mm.tile([128, 128], F32, tag="mm")
                    nc.tensor.matmul(out=zps[:, 0:2 * H2], lhsT=zT[:],
                                     rhs=W2x2_sb[:], start=True, stop=True)
                    zfin = ep.tile([128, 2 * H2], F32, tag="zfin")
                    nc.vector.tensor_add(out=zfin[:, :ntl * H2],
                                         in0=zps[:, :ntl * H2],
                                         in1=b2x2_sb[:, :ntl * H2])
                    nc.vector.tensor_add(out=zfin[:, :ntl * H2],
                                         in0=zfin[:, :ntl * H2],
                                         in1=r2_store[:, t * H2:(t + ntl) * H2])
                    for a in range(ntl):
                        tt = t + a
                        zext = ep.tile([128, 17], F32, tag="zext")
                        nc.vector.tensor_copy(out=zext[:, 0:H2],
                                              in_=zfin[:, a * H2:(a + 1) * H2])
                        nc.vector.memset(zext[:, H2:], 1.0)
                        k = tcount[0]
                        s0 = ep.tile([128, 128], F32, tag="s0")
                        nc.vector.tensor_tensor(
                            out=s0[:],
                            in0=batch_sb[:, tt:tt + 1].to_broadcast([128, 128]),
                            in1=iotaf_sb[:, 0:128], op=ISEQ)
                        nc.tensor.matmul(out=pool_ps[:, 0:17], lhsT=s0[:],
                                         rhs=zext[:], start=False,
                                         stop=(k == NT - 1),
                                         skip_group_check=True)
                        s1 = ep.tile([128, 128], F32, tag="s1")
                        nc.vector.tensor_tensor(
                            out=s1[:],
                            in0=batch_sb[:, tt:tt + 1].to_broadcast([128, 128]),
                            in1=iotaf_sb[:, 128:256], op=ISEQ)
                        nc.tensor.matmul(out=pool_ps[:, 17:34], lhsT=s1[:],
                                         rhs=zext[:], start=False,
                                         stop=(k == NT - 1),
                                         skip_group_check=True)
                        tcount[0] += 1

                run_layer(1)
                nc.vector.memset(pool_ps[:], 0.0)
                run_layer(2)

                psums = ep.tile([128, 34], F32, tag="psums")
                nc.vector.tensor_copy(out=psums[:], in_=pool_ps[:])
                nc.sync.dma_start(out=pool_in[0:128, :], in_=psums[:, 0:17])
                nc.sync.dma_start(out=pool_in[128:256, :], in_=psums[:, 17:34])

                nc.gpsimd.collective_compute(
                    "AllReduce", mybir.AluOpType.add,
                    replica_groups=[list(range(NCORES))],
                    ins=[pool_in.opt()], outs=[pool_out.opt()])

                # ------------- classifier head (two graph windows) --------
                for wdw in range(2):
                    sums = ep.tile([128, 17], F32, tag="hsum")
                    nc.sync.dma_start(out=sums[:],
                                      in_=pool_out[wdw * 128:(wdw + 1) * 128, :])
                    cnt = ep.tile([128, 1], F32, tag="hcnt")
                    nc.vector.tensor_scalar_max(out=cnt[:], in0=sums[:, 16:17],
                                                scalar1=1.0)
                    rec = ep.tile([128, 1], F32, tag="hrec")
                    nc.vector.reciprocal(out=rec[:], in_=cnt[:])
                    ge = ep.tile([128, 16], F32, tag="hge")
                    nc.vector.tensor_scalar_mul(out=ge[:], in0=sums[:, :16],
                                                scalar1=rec[:])
                    geT_ps = ps_tp.tile([128, 128], F32, tag="tp")
                    nc.tensor.transpose(out=geT_ps[:16, :], in_=ge[:],
                                        identity=ident_sb[:])
                    geT = ep.tile([16, 128], F32, tag="hget")
                    nc.vector.tensor_copy(out=geT[:], in_=geT_ps[:16, :])
                    u_ps = ps_mm.tile([128, 128], F32, tag="mm")
                    nc.tensor.matmul(out=u_ps[:, 0:80], lhsT=geT[:],
                                     rhs=Wf1t_sb[:], start=True, stop=False)
                    nc.tensor.matmul(out=u_ps[:, 0:80], lhsT=ones1[:],
                                     rhs=mc_sb[:], start=False, stop=True)
                    u = ep.tile([128, 80], F32, tag="hu")
                    nc.scalar.activation(out=u[:], in_=u_ps[:, 0:80], func=SILU)
                    uT_ps = ps_tp.tile([128, 128], F32, tag="tp")
                    nc.tensor.transpose(out=uT_ps[:80, :], in_=u[:],
                                        identity=ident_sb[:])
                    uT = ep.tile([80, 128], F32, tag="hut")
                    nc.vector.tensor_copy(out=uT[:], in_=uT_ps[:80, :])
                    o_ps = ps_mm.tile([128, 128], F32, tag="mm")
                    nc.tensor.matmul(out=o_ps[:, 0:NCLS], lhsT=uT[:],
                                     rhs=Wf2_sb[:], start=True, stop=False)
                    nc.tensor.matmul(out=o_ps[:, 0:NCLS], lhsT=ones1[:],
                                     rhs=bf2_sb[:], start=False, stop=True)
                    o = ep.tile([128, NCLS], F32, tag="ho")
                    nc.vector.tensor_copy(out=o[:], in_=o_ps[:, 0:NCLS])
                    nc.sync.dma_start(out=out[wdw * 128:(wdw + 1) * 128, :],
                                      in_=o[:])

    nc.compile()
    return nc


def _host_metrics_contrib(tolerance, cost, time, quantity,
                          mW1, mb1, mW2, mb2, Wf1, bf1):
    silu = lambda v: v / (1.0 + np.exp(-v))
    m = np.stack([np.asarray(v, np.float32).reshape(1, 1) for v in
                  (tolerance, cost, time, quantity)])         # [4,1,1]
    e = silu(np.einsum('gij,gjk->gik', m, np.asarray(mW1, np.float32))
             + np.asarray(mb1, np.float32)[:, None, :])
    e = (np.einsum('gij,gjk->gik', e, np.asarray(mW2, np.float32))
         + np.asarray(mb2, np.float32)[:, None, :])           # [4,1,16]
    metvec = e.transpose(1, 0, 2).reshape(1, 64)
    mc = metvec @ np.asarray(Wf1, np.float32)[16:, :] + np.asarray(bf1, np.float32)[None, :]
    return mc.astype(np.float32)


def _blockdiag2(W):
    """[64,16] -> [128,32] blockdiag bf16."""
    out = np.zeros((128, 32), np.float32)
    out[:64, :16] = W
    out[64:, 16:] = W
    return out.astype(BF16)


def kernel(x, edge_index, batch, tolerance, cost, time, quantity,
           W1, b1, W2, b2, Wr1, br1, Wr2, br2, alpha1, alpha2,
           mW1, mb1, mW2, mb2, Wf1, bf1, Wf2, bf2):
    x = np.asarray(x, np.float32)
    src = np.asarray(edge_index[0], np.int64)
    dst = np.asarray(edge_index[1], np.int64)
    batch = np.asarray(batch, np.int64)

    deg = 1.0 + np.bincount(dst, minlength=N).astype(np.float32)
    dinv_full = 1.0 / np.sqrt(deg)

    sched, per_core = _build_structure(src, dst)
    nc = _build_program(sched, float(alpha1), float(alpha2))

    W1f = np.asarray(W1, np.float32)
    Wr1f = np.asarray(Wr1, np.float32)
    Wcat = np.concatenate([W1f, Wr1f], axis=1).astype(BF16)   # [128,128]
    b1f = np.asarray(b1, np.float32)
    common = {
        "iotab": np.tile(np.arange(128, dtype=np.float32), (128, 1)).astype(BF16),
        "iotaf": np.tile(np.arange(256, dtype=np.float32), (128, 1)),
        "ident": np.eye(128, dtype=np.float32),
        "identb": np.eye(128, dtype=np.float32).astype(BF16),
        "Wcat": Wcat,
        "Wr2x2": _blockdiag2(np.asarray(Wr2, np.float32)),
        "W2x2": _blockdiag2(np.asarray(W2, np.float32)),
        "b1b2": np.tile(np.concatenate([b1f, b1f])[None, :], (128, 1)),
        "br1b": np.tile(np.asarray(br1, np.float32), (128, 1)),
        "br2x2": np.tile(np.concatenate([np.asarray(br2, np.float32)] * 2)[None, :], (128, 1)),
        "b2x2": np.tile(np.concatenate([np.asarray(b2, np.float32)] * 2)[None, :], (128, 1)),
        "Wf1t": np.asarray(Wf1[:16, :], np.float32),
        "Wf2": np.asarray(Wf2, np.float32),
        "bf2r": np.asarray(bf2, np.float32)[None, :],
        "mcin": _host_metrics_contrib(tolerance, cost, time, quantity,
                                      mW1, mb1, mW2, mb2, Wf1, bf1),
    }

    in_maps = []
    for c in range(NCORES):
        lo, hi = c * SHARD, (c + 1) * SHARD
        xs = np.zeros((SHARD_PAD, CIN), np.float32)
        xs[:SHARD] = x[lo:hi]
        dv = np.zeros(SHARD_PAD, np.float32)
        dv[:SHARD] = dinv_full[lo:hi]
        bf_loc = np.full(SHARD_PAD, -1.0, np.float32)
        bf_loc[:SHARD] = batch[lo:hi].astype(np.float32)
        m = dict(common)
        m["xT"] = np.ascontiguousarray(xs.T).astype(BF16)
        m["dinv"] = dv.reshape(NT, 128).T.copy()
        m["dinv2"] = (dv * dv).reshape(NT, 128).T.copy()
        m["batchf"] = bf_loc.reshape(NT, 128).T.copy()
        m["idx"] = per_core[c]["idx"]
        m["pcol"] = per_core[c]["pcol"]
        in_maps.append(m)

    res = run_bass_kernel_spmd(nc, in_maps, list(range(NCORES)))
    kernel._last = (nc, in_maps)   # for external profiling harnesses
    kernel._res = res
    return np.asarray(res.results[0]["out"], np.float32)
